# revision 12
# baseline (speedup 1.0000x reference)
"""Trainium2 Bass kernel for nn_AngularHallucination.

Math reduction: with D=1 the affine-grid + trilinear grid_sample collapses to a
per-column 1-D lerp along W (y maps to identity, z contributes a tent weight),
i.e. x_angular = x @ A for a fixed 256x256 matrix A (2 nonzeros per column)
computable on the host from `angle`.  Per batch b:
    corr  = X @ (X A)^T          (64x64, contraction over N = H*W = 65536)
    pro   = softmax(corr, dim=0-of-columns)  (column softmax, near-one-hot)
    out   = pro @ X
Pure data-parallel over B=8: one batch per NeuronCore.
"""

import math

import numpy as np

import concourse.bass as bass
import concourse.tile as tile
from concourse import bacc, mybir
from concourse.bass_utils import run_bass_kernel_spmd
from concourse.masks import make_identity

F32 = mybir.dt.float32

B, C, H, W = 8, 64, 256, 256
HH = 2              # h halves (h = hh*128 + hp)
HP = H // HH        # 128
P = 128
N_CORES = 8
HB = 8              # h' rows per block
NBLK = HP // HB     # 16 blocks


def _build_A(angle: int) -> np.ndarray:
    """A[v, w'] so that x_angular[..., w'] = sum_v x[..., v] * A[v, w'].

    Mirrors reference.py's float32 arithmetic exactly (affine_grid with
    align_corners=False + trilinear grid_sample with zeros padding, D=1).
    """
    D = 1
    c = math.cos(angle)
    s = math.sin(angle)
    xs = ((2.0 * np.arange(W, dtype=np.float32) + 1.0) / W - 1.0).astype(np.float32)
    gx = np.float32(c) * xs
    gz = np.float32(-s) * xs
    ix = ((gx + np.float32(1.0)) * W - np.float32(1.0)) * np.float32(0.5)
    iz = ((gz + np.float32(1.0)) * D - np.float32(1.0)) * np.float32(0.5)
    ix0 = np.floor(ix)
    iz0 = np.floor(iz)
    fx = ix - ix0
    fz = iz - iz0
    ix0 = ix0.astype(np.int64)
    iz0 = iz0.astype(np.int64)
    A = np.zeros((W, W), dtype=np.float64)
    for w in range(W):
        for dz in (0, 1):
            zi = iz0[w] + dz
            wz = fz[w] if dz else 1.0 - fz[w]
            if not (0 <= zi < D):
                continue
            for dx in (0, 1):
                xi = ix0[w] + dx
                wx = fx[w] if dx else 1.0 - fx[w]
                if not (0 <= xi < W):
                    continue
                A[xi, w] += np.float64(wz) * np.float64(wx)
    return A.astype(np.float32)


def _kernel_body(nc, tc, x_d, a_d, out_d, ctx, dbg=None):
    # partition layout p = hh*64 + c ; free = (hp, w)
    x_ap = x_d.ap()      # [C, H, W]
    out_ap = out_d.ap()  # [C, H, W]

    consts = ctx.enter_context(tc.tile_pool(name="consts", bufs=1))
    small = ctx.enter_context(tc.tile_pool(name="small", bufs=1))

    A_sb = consts.tile([P, 2, W], F32, name="A_sb")  # [v127, vh, w']
    nc.sync.dma_start(out=A_sb, in_=a_d.ap().rearrange("(vh v) w -> v vh w", vh=2))
    ident = consts.tile([P, P], F32, name="ident")
    make_identity(nc, ident)

    xpool = ctx.enter_context(tc.tile_pool(name="x", bufs=NBLK))
    xblks = []
    for b in range(NBLK):
        xb = xpool.tile([P, HB, W], F32, tag="xblk")
        for hh in range(HH):
            nc.sync.dma_start(
                out=xb[hh * 64 : (hh + 1) * 64, :, :],
                in_=x_ap[:, hh * HP + b * HB : hh * HP + (b + 1) * HB, :],
            )
        xblks.append(xb)

    # ---------------- phase 1: transposes, x_angular, corr ----------------
    n_corr = NBLK * HB * 2 * 2
    i_corr = 0
    with (
        tc.tile_pool(name="xt", bufs=2) as xtpool,
        tc.tile_pool(name="yt", bufs=2) as ytpool,
        tc.tile_pool(name="pt", bufs=2, space="PSUM") as ptpool,
        tc.tile_pool(name="py", bufs=2, space="PSUM") as pypool,
        tc.tile_pool(name="pcorr", bufs=1, space="PSUM") as pcpool,
    ):
        corr_ps = pcpool.tile([64, 64], F32, name="corr_ps")
        drain_i = 0
        for b in range(NBLK):
            xb = xblks[b]
            # xt/yt: [w127, i_h, wh, (hh c)]
            xt = xtpool.tile([P, HB, 2, P], F32, tag="xt")
            yt = ytpool.tile([P, HB, 2, P], F32, tag="yt")

            # transposes: Xt[w, (hh c)] = matmul(lhsT=x-slab, rhs=I)
            for g in range(HB * 2 // 4):  # groups of 4 -> one psum bank
                ps = ptpool.tile([P, 2, 2, P], F32, tag="ps_t")
                for k in range(4):
                    idx = g * 4 + k
                    i_h, wh = divmod(idx, 2)
                    nc.tensor.matmul(
                        ps[:, k // 2, k % 2, :],
                        lhsT=xb[:, i_h, bass.ts(wh, P)],
                        rhs=ident,
                        start=True,
                        stop=True,
                    )
                eng = nc.scalar if (drain_i % 2) else nc.vector
                drain_i += 1
                cp = eng.copy if eng is nc.scalar else eng.tensor_copy
                cp(out=xt[:, 2 * g : 2 * g + 2, :, :], in_=ps)

            # x_angular^T: Yt[w', (i_h, hh c)] = sum_v A[v, w'] Xt[v, (i_h, hh c)]
            for whp in range(2):
                for ch in range(HB // 4):  # psum chunks of [128, 4*128]
                    ps = pypool.tile([P, 4, P], F32, tag="ps_y")
                    for vh in range(2):
                        nc.tensor.matmul(
                            ps,
                            lhsT=A_sb[:, vh, bass.ts(whp, P)],
                            rhs=xt[:, 4 * ch : 4 * ch + 4, vh, :],
                            start=(vh == 0),
                            stop=(vh == 1),
                        )
                    eng = nc.scalar if (drain_i % 2) else nc.vector
                    drain_i += 1
                    cp = eng.copy if eng is nc.scalar else eng.tensor_copy
                    cp(out=yt[:, 4 * ch : 4 * ch + 4, whp, :], in_=ps)

            # corr^T[j, i] += Yt-chunk^T @ Xt-chunk
            for i_h in range(HB):
                for wh in range(2):
                    for hh in range(2):
                        nc.tensor.matmul(
                            corr_ps,
                            lhsT=yt[:, i_h, wh, hh * 64 : (hh + 1) * 64],
                            rhs=xt[:, i_h, wh, hh * 64 : (hh + 1) * 64],
                            start=(i_corr == 0),
                            stop=(i_corr == n_corr - 1),
                        )
                        i_corr += 1

            if dbg is not None and b == 0:
                nc.sync.dma_start(out=dbg["xt0"].ap(), in_=xt)
                nc.sync.dma_start(out=dbg["yt0"].ap(), in_=yt)

        # ---------------- softmax over free axis of corrT ----------------
        corr_sb = small.tile([64, 64], F32, name="corr_sb")
        nc.vector.tensor_copy(out=corr_sb, in_=corr_ps)
        if dbg is not None:
            nc.sync.dma_start(out=dbg["corr"].ap(), in_=corr_sb)

    negm = small.tile([64, 1], F32, name="negm")
    nc.vector.tensor_reduce(
        out=negm, in_=corr_sb, axis=mybir.AxisListType.X,
        op=mybir.AluOpType.max, negate=True,
    )
    proT = small.tile([64, 64], F32, name="proT")
    nc.scalar.activation(
        out=proT, in_=corr_sb, func=mybir.ActivationFunctionType.Exp,
        bias=negm, scale=1.0,
    )
    ssum = small.tile([64, 1], F32, name="ssum")
    nc.vector.reduce_sum(out=ssum, in_=proT, axis=mybir.AxisListType.X)
    rinv = small.tile([64, 1], F32, name="rinv")
    nc.vector.reciprocal(out=rinv, in_=ssum)
    nc.vector.tensor_scalar_mul(out=proT, in0=proT, scalar1=rinv)
    if dbg is not None:
        nc.sync.dma_start(out=dbg["proT"].ap(), in_=proT)

    # block-diagonal [proT 0; 0 proT] (stage-C stationary weights)
    proT_bd = consts.tile([P, P], F32, name="proT_bd")
    nc.vector.memset(proT_bd, 0.0)
    nc.vector.tensor_copy(out=proT_bd[0:64, 0:64], in_=proT)
    # cross-partition move -> SBUF-to-SBUF DMA
    nc.sync.dma_start(out=proT_bd[64:128, 64:128], in_=proT)

    # ---------------- phase C: out = pro @ X ----------------
    OB = 4  # h'-pairs per out DMA batch
    with (
        tc.tile_pool(name="po", bufs=2, space="PSUM") as popool,
        tc.tile_pool(name="outb", bufs=2) as outpool,
    ):
        drain_i = 0
        for b in range(NBLK):
            xb = xblks[b]
            for ob in range(HB // 2 // OB):  # out batches within block
                osb = outpool.tile([P, OB, 2, W], F32, tag="osb")
                for q in range(OB):
                    hp0 = (ob * OB + q) * 2
                    ps = popool.tile([P, 2, W], F32, tag="ps_o")
                    nc.tensor.matmul(
                        ps,
                        lhsT=proT_bd,
                        rhs=xb[:, hp0 : hp0 + 2, :],
                        start=True,
                        stop=True,
                    )
                    eng = nc.scalar if (drain_i % 2) else nc.vector
                    drain_i += 1
                    cp = eng.copy if eng is nc.scalar else eng.tensor_copy
                    cp(out=osb[:, q, :, :], in_=ps)
                hpb = b * HB + ob * OB * 2
                for hh in range(HH):
                    nc.sync.dma_start(
                        out=out_ap[:, hh * HP + hpb : hh * HP + hpb + OB * 2, :],
                        in_=osb[hh * 64 : (hh + 1) * 64].rearrange(
                            "p q t w -> p (q t) w"
                        ),
                    )


_GRAPH_CACHE = {}


def _get_graph(angle: int, debug_probes: bool = False):
    key = (angle, debug_probes)
    if key in _GRAPH_CACHE:
        return _GRAPH_CACHE[key]
    nc = bacc.Bacc(
        "TRN2",
        target_bir_lowering=False,
        debug=False,
        enable_asserts=False,
        num_devices=N_CORES,
    )
    x_d = nc.dram_tensor("x", [C, H, W], F32, kind="ExternalInput")
    a_d = nc.dram_tensor("amat", [W, W], F32, kind="ExternalInput")
    out_d = nc.dram_tensor("out", [C, H, W], F32, kind="ExternalOutput")
    dbg = None
    if debug_probes:
        dbg = {
            "xt0": nc.dram_tensor("dbg_xt0", [P, HB, 2, P], F32, kind="ExternalOutput"),
            "yt0": nc.dram_tensor("dbg_yt0", [P, HB, 2, P], F32, kind="ExternalOutput"),
            "corr": nc.dram_tensor("dbg_corr", [64, 64], F32, kind="ExternalOutput"),
            "proT": nc.dram_tensor("dbg_proT", [64, 64], F32, kind="ExternalOutput"),
        }
    from contextlib import ExitStack

    with tile.TileContext(nc) as tc, ExitStack() as ctx:
        _kernel_body(nc, tc, x_d, a_d, out_d, ctx, dbg=dbg)
    nc.compile()
    _GRAPH_CACHE[key] = nc
    return nc


def _run(x_cls: np.ndarray, angle: int, **spmd_kwargs):
    x_cls = np.ascontiguousarray(np.asarray(x_cls, dtype=np.float32))
    assert x_cls.shape == (B, C, H, W), x_cls.shape
    A = _build_A(int(angle))
    nc = _get_graph(int(angle))
    in_maps = [{"x": x_cls[i], "amat": A} for i in range(N_CORES)]
    res = run_bass_kernel_spmd(nc, in_maps, core_ids=list(range(N_CORES)), **spmd_kwargs)
    out = np.stack([r["out"] for r in res.results], axis=0)
    return out, res


def kernel(x_cls, angle):
    out, _ = _run(x_cls, int(np.asarray(angle)))
    return out


# revision 24
# speedup vs baseline: 1.6611x; 1.6611x over previous
"""Trainium2 Bass kernel for nn_AngularHallucination.

Math reduction: with D=1 the affine-grid + trilinear grid_sample collapses to a
per-column 1-D lerp along W (y maps to identity, z contributes a tent weight),
i.e. x_angular = x @ A for a fixed 256x256 matrix A (2 nonzeros per column)
computable on the host from `angle`.  Per batch b:
    corr  = X @ (X A)^T          (64x64, contraction over N = H*W = 65536)
    pro   = softmax(corr, dim=0-of-columns)  (column softmax, near-one-hot)
    out   = pro @ X
Pure data-parallel over B=8: one batch per NeuronCore.
"""

import math

import numpy as np

import concourse.bass as bass
import concourse.tile as tile
from concourse import bacc, mybir
from concourse.bass_utils import run_bass_kernel_spmd
from concourse.masks import make_identity

F32 = mybir.dt.float32
F32R = mybir.dt.float32r

B, C, H, W = 8, 64, 256, 256
HH = 2              # h halves (h = hh*128 + hp)
HP = H // HH        # 128
P = 128
N_CORES = 8
HB = 8              # h' rows per block
NBLK = HP // HB     # 16 blocks


def _build_A(angle: int) -> np.ndarray:
    """A[v, w'] so that x_angular[..., w'] = sum_v x[..., v] * A[v, w'].

    Mirrors reference.py's float32 arithmetic exactly (affine_grid with
    align_corners=False + trilinear grid_sample with zeros padding, D=1).
    """
    D = 1
    c = math.cos(angle)
    s = math.sin(angle)
    xs = ((2.0 * np.arange(W, dtype=np.float32) + 1.0) / W - 1.0).astype(np.float32)
    gx = np.float32(c) * xs
    gz = np.float32(-s) * xs
    ix = ((gx + np.float32(1.0)) * W - np.float32(1.0)) * np.float32(0.5)
    iz = ((gz + np.float32(1.0)) * D - np.float32(1.0)) * np.float32(0.5)
    ix0 = np.floor(ix)
    iz0 = np.floor(iz)
    fx = ix - ix0
    fz = iz - iz0
    ix0 = ix0.astype(np.int64)
    iz0 = iz0.astype(np.int64)
    A = np.zeros((W, W), dtype=np.float64)
    for w in range(W):
        for dz in (0, 1):
            zi = iz0[w] + dz
            wz = fz[w] if dz else 1.0 - fz[w]
            if not (0 <= zi < D):
                continue
            for dx in (0, 1):
                xi = ix0[w] + dx
                wx = fx[w] if dx else 1.0 - fx[w]
                if not (0 <= xi < W):
                    continue
                A[xi, w] += np.float64(wz) * np.float64(wx)
    return A.astype(np.float32)


def _kernel_body(nc, tc, x_d, a_d, out_d, ctx, dbg=None):
    # partition layout p = hh*64 + c ; free = (hp, w)
    x_ap = x_d.ap()      # [C, H, W]
    out_ap = out_d.ap()  # [C, H, W]

    consts = ctx.enter_context(tc.tile_pool(name="consts", bufs=1))
    small = ctx.enter_context(tc.tile_pool(name="small", bufs=1))

    A_sb = consts.tile([P, 2, W], F32, name="A_sb")  # [v127, vh, w']
    nc.sync.dma_start(out=A_sb, in_=a_d.ap().rearrange("(vh v) w -> v vh w", vh=2))
    A_sbr = consts.tile([P, 2, W], F32R, name="A_sbr")
    nc.vector.tensor_copy(out=A_sbr, in_=A_sb)
    ident = consts.tile([P, P], F32, name="ident")
    make_identity(nc, ident)

    xpool = ctx.enter_context(tc.tile_pool(name="x", bufs=NBLK))
    xblks = []
    for b in range(NBLK):
        xb = xpool.tile([P, HB, W], F32, tag="xblk")
        for hh in range(HH):
            nc.sync.dma_start(
                out=xb[hh * 64 : (hh + 1) * 64, :, :],
                in_=x_ap[:, hh * HP + b * HB : hh * HP + (b + 1) * HB, :],
            )
        xblks.append(xb)

    # ---------------- phase 1: transposes, x_angular, corr ----------------
    n_corr = NBLK * HB * 2 * 2
    i_corr = 0
    with (
        tc.tile_pool(name="xt", bufs=2) as xtpool,
        tc.tile_pool(name="yt", bufs=2) as ytpool,
        tc.tile_pool(name="pt", bufs=2, space="PSUM") as ptpool,
        tc.tile_pool(name="py", bufs=2, space="PSUM") as pypool,
        tc.tile_pool(name="pcorr", bufs=1, space="PSUM") as pcpool,
    ):
        corr_ps = pcpool.tile([64, 64], F32, name="corr_ps")
        drain_i = 0
        for b in range(NBLK):
            xb = xblks[b]
            # xt/yt: [w127, i_h, wh, (hh c)]
            xt = xtpool.tile([P, HB, 2, P], F32R, tag="xt")
            yt = ytpool.tile([P, HB, 2, P], F32R, tag="yt")

            # transposes: Xt[w, (hh c)] = x-slab^T (PE transpose mode)
            for g in range(HB * 2 // 4):  # groups of 4 -> one psum bank
                ps = ptpool.tile([P, 2, 2, P], F32, tag="ps_t")
                for k in range(4):
                    idx = g * 4 + k
                    i_h, wh = divmod(idx, 2)
                    nc.tensor.transpose(
                        ps[:, k // 2, k % 2, :],
                        xb[:, i_h, bass.ts(wh, P)],
                        ident,
                    )
                eng = nc.scalar if (drain_i % 2) else nc.vector
                drain_i += 1
                cp = eng.copy if eng is nc.scalar else eng.tensor_copy
                cp(out=xt[:, 2 * g : 2 * g + 2, :, :], in_=ps)

            # x_angular^T: Yt[w', (i_h, hh c)] = sum_v A[v, w'] Xt[v, (i_h, hh c)]
            for whp in range(2):
                for ch in range(HB // 4):  # psum chunks of [128, 4*128]
                    ps = pypool.tile([P, 4, P], F32, tag="ps_y")
                    for vh in range(2):
                        nc.tensor.matmul(
                            ps,
                            lhsT=A_sbr[:, vh, bass.ts(whp, P)],
                            rhs=xt[:, 4 * ch : 4 * ch + 4, vh, :],
                            start=(vh == 0),
                            stop=(vh == 1),
                        )
                    eng = nc.scalar if (drain_i % 2) else nc.vector
                    drain_i += 1
                    cp = eng.copy if eng is nc.scalar else eng.tensor_copy
                    cp(out=yt[:, 4 * ch : 4 * ch + 4, whp, :], in_=ps)

            # corr^T[j, i] += Yt-chunk^T @ Xt-chunk
            for i_h in range(HB):
                for wh in range(2):
                    for hh in range(2):
                        nc.tensor.matmul(
                            corr_ps,
                            lhsT=yt[:, i_h, wh, hh * 64 : (hh + 1) * 64],
                            rhs=xt[:, i_h, wh, hh * 64 : (hh + 1) * 64],
                            start=(i_corr == 0),
                            stop=(i_corr == n_corr - 1),
                        )
                        i_corr += 1

            if dbg is not None and b == 0:
                nc.sync.dma_start(out=dbg["xt0"].ap(), in_=xt.bitcast(F32))
                nc.sync.dma_start(out=dbg["yt0"].ap(), in_=yt.bitcast(F32))

        # ---------------- softmax over free axis of corrT ----------------
        corr_sb = small.tile([64, 64], F32, name="corr_sb")
        nc.vector.tensor_copy(out=corr_sb, in_=corr_ps)
        if dbg is not None:
            nc.sync.dma_start(out=dbg["corr"].ap(), in_=corr_sb)

    negm = small.tile([64, 1], F32, name="negm")
    nc.vector.tensor_reduce(
        out=negm, in_=corr_sb, axis=mybir.AxisListType.X,
        op=mybir.AluOpType.max, negate=True,
    )
    proT = small.tile([64, 64], F32, name="proT")
    nc.scalar.activation(
        out=proT, in_=corr_sb, func=mybir.ActivationFunctionType.Exp,
        bias=negm, scale=1.0,
    )
    ssum = small.tile([64, 1], F32, name="ssum")
    nc.vector.reduce_sum(out=ssum, in_=proT, axis=mybir.AxisListType.X)
    rinv = small.tile([64, 1], F32, name="rinv")
    nc.vector.reciprocal(out=rinv, in_=ssum)
    nc.vector.tensor_scalar_mul(out=proT, in0=proT, scalar1=rinv)
    if dbg is not None:
        nc.sync.dma_start(out=dbg["proT"].ap(), in_=proT)

    # block-diagonal [proT 0; 0 proT] (stage-C stationary weights)
    proT_bd = consts.tile([P, P], F32, name="proT_bd")
    nc.vector.memset(proT_bd, 0.0)
    nc.vector.tensor_copy(out=proT_bd[0:64, 0:64], in_=proT)
    # cross-partition move -> SBUF-to-SBUF DMA
    nc.sync.dma_start(out=proT_bd[64:128, 64:128], in_=proT)

    # ---------------- phase C: out = pro @ X ----------------
    OB = 4  # h'-pairs per out DMA batch
    with (
        tc.tile_pool(name="po", bufs=2, space="PSUM") as popool,
        tc.tile_pool(name="outb", bufs=2) as outpool,
    ):
        drain_i = 0
        for b in range(NBLK):
            xb = xblks[b]
            for ob in range(HB // 2 // OB):  # out batches within block
                osb = outpool.tile([P, OB, 2, W], F32, tag="osb")
                for q in range(OB):
                    hp0 = (ob * OB + q) * 2
                    ps = popool.tile([P, 2, W], F32, tag="ps_o")
                    nc.tensor.matmul(
                        ps,
                        lhsT=proT_bd,
                        rhs=xb[:, hp0 : hp0 + 2, :],
                        start=True,
                        stop=True,
                    )
                    eng = nc.scalar if (drain_i % 2) else nc.vector
                    drain_i += 1
                    cp = eng.copy if eng is nc.scalar else eng.tensor_copy
                    cp(out=osb[:, q, :, :], in_=ps)
                hpb = b * HB + ob * OB * 2
                for hh in range(HH):
                    nc.sync.dma_start(
                        out=out_ap[:, hh * HP + hpb : hh * HP + hpb + OB * 2, :],
                        in_=osb[hh * 64 : (hh + 1) * 64].rearrange(
                            "p q t w -> p (q t) w"
                        ),
                    )


_GRAPH_CACHE = {}


def _get_graph(angle: int, debug_probes: bool = False):
    key = (angle, debug_probes)
    if key in _GRAPH_CACHE:
        return _GRAPH_CACHE[key]
    nc = bacc.Bacc(
        "TRN2",
        target_bir_lowering=False,
        debug=False,
        enable_asserts=False,
        num_devices=N_CORES,
    )
    x_d = nc.dram_tensor("x", [C, H, W], F32, kind="ExternalInput")
    a_d = nc.dram_tensor("amat", [W, W], F32, kind="ExternalInput")
    out_d = nc.dram_tensor("out", [C, H, W], F32, kind="ExternalOutput")
    dbg = None
    if debug_probes:
        dbg = {
            "xt0": nc.dram_tensor("dbg_xt0", [P, HB, 2, P], F32, kind="ExternalOutput"),
            "yt0": nc.dram_tensor("dbg_yt0", [P, HB, 2, P], F32, kind="ExternalOutput"),
            "corr": nc.dram_tensor("dbg_corr", [64, 64], F32, kind="ExternalOutput"),
            "proT": nc.dram_tensor("dbg_proT", [64, 64], F32, kind="ExternalOutput"),
        }
    from contextlib import ExitStack

    with tile.TileContext(nc) as tc, ExitStack() as ctx:
        _kernel_body(nc, tc, x_d, a_d, out_d, ctx, dbg=dbg)
    nc.compile()
    _GRAPH_CACHE[key] = nc
    return nc


def _run(x_cls: np.ndarray, angle: int, **spmd_kwargs):
    x_cls = np.ascontiguousarray(np.asarray(x_cls, dtype=np.float32))
    assert x_cls.shape == (B, C, H, W), x_cls.shape
    A = _build_A(int(angle))
    nc = _get_graph(int(angle))
    in_maps = [{"x": x_cls[i], "amat": A} for i in range(N_CORES)]
    res = run_bass_kernel_spmd(nc, in_maps, core_ids=list(range(N_CORES)), **spmd_kwargs)
    out = np.stack([r["out"] for r in res.results], axis=0)
    return out, res


def kernel(x_cls, angle):
    out, _ = _run(x_cls, int(np.asarray(angle)))
    return out


# revision 27
# speedup vs baseline: 2.1155x; 1.2736x over previous
"""Trainium2 Bass kernel for nn_AngularHallucination.

Math reduction: with D=1 the affine-grid + trilinear grid_sample collapses to a
per-column 1-D lerp along W (y maps to identity, z contributes a tent weight),
i.e. x_angular = x @ A for a fixed 256x256 matrix A (2 nonzeros per column)
computable on the host from `angle`.  Per batch b:
    corr  = X @ (X A)^T          (64x64, contraction over N = H*W = 65536)
    pro   = softmax(corr, dim=0-of-columns)  (column softmax, near-one-hot)
    out   = pro @ X
Pure data-parallel over B=8: one batch per NeuronCore.
"""

import math

import numpy as np

import concourse.bass as bass
import concourse.tile as tile
from concourse import bacc, mybir
from concourse.bass_utils import run_bass_kernel_spmd
from concourse.masks import make_identity

F32 = mybir.dt.float32
F32R = mybir.dt.float32r

B, C, H, W = 8, 64, 256, 256
HH = 2              # h halves (h = hh*128 + hp)
HP = H // HH        # 128
P = 128
N_CORES = 8
HB = 8              # h' rows per block
NBLK = HP // HB     # 16 blocks


def _build_A(angle: int) -> np.ndarray:
    """A[v, w'] so that x_angular[..., w'] = sum_v x[..., v] * A[v, w'].

    Mirrors reference.py's float32 arithmetic exactly (affine_grid with
    align_corners=False + trilinear grid_sample with zeros padding, D=1).
    """
    D = 1
    c = math.cos(angle)
    s = math.sin(angle)
    xs = ((2.0 * np.arange(W, dtype=np.float32) + 1.0) / W - 1.0).astype(np.float32)
    gx = np.float32(c) * xs
    gz = np.float32(-s) * xs
    ix = ((gx + np.float32(1.0)) * W - np.float32(1.0)) * np.float32(0.5)
    iz = ((gz + np.float32(1.0)) * D - np.float32(1.0)) * np.float32(0.5)
    ix0 = np.floor(ix)
    iz0 = np.floor(iz)
    fx = ix - ix0
    fz = iz - iz0
    ix0 = ix0.astype(np.int64)
    iz0 = iz0.astype(np.int64)
    A = np.zeros((W, W), dtype=np.float64)
    for w in range(W):
        for dz in (0, 1):
            zi = iz0[w] + dz
            wz = fz[w] if dz else 1.0 - fz[w]
            if not (0 <= zi < D):
                continue
            for dx in (0, 1):
                xi = ix0[w] + dx
                wx = fx[w] if dx else 1.0 - fx[w]
                if not (0 <= xi < W):
                    continue
                A[xi, w] += np.float64(wz) * np.float64(wx)
    return A.astype(np.float32)


def _kernel_body(nc, tc, x_d, a_d, out_d, ctx, dbg=None):
    # partition layout p = hh*64 + c ; free = (hp, w)
    x_ap = x_d.ap()      # [C, H, W]
    out_ap = out_d.ap()  # [C, H, W]

    consts = ctx.enter_context(tc.tile_pool(name="consts", bufs=1))
    small = ctx.enter_context(tc.tile_pool(name="small", bufs=1))

    A_sb = consts.tile([P, 2, W], F32, name="A_sb")  # [v127, vh, w']
    nc.sync.dma_start(out=A_sb, in_=a_d.ap().rearrange("(vh v) w -> v vh w", vh=2))
    A_sbr = consts.tile([P, 2, W], F32R, name="A_sbr")
    nc.vector.tensor_copy(out=A_sbr, in_=A_sb)
    ident = consts.tile([P, P], F32, name="ident")
    make_identity(nc, ident)
    identr = consts.tile([P, P], F32R, name="identr")
    nc.vector.tensor_copy(out=identr, in_=ident)

    xpool = ctx.enter_context(tc.tile_pool(name="x", bufs=NBLK))
    xblks = []
    for b in range(NBLK):
        xb = xpool.tile([P, HB, W], F32R, tag="xblk")
        for hh in range(HH):
            nc.gpsimd.dma_start(
                out=xb[hh * 64 : (hh + 1) * 64, :, :],
                in_=x_ap[:, hh * HP + b * HB : hh * HP + (b + 1) * HB, :],
            )
        xblks.append(xb)

    # ---------------- phase 1: transposes, x_angular, corr ----------------
    n_corr = NBLK * HB * 2 * 2
    i_corr = 0
    with (
        tc.tile_pool(name="xt", bufs=2) as xtpool,
        tc.tile_pool(name="yt", bufs=2) as ytpool,
        tc.tile_pool(name="pt", bufs=2, space="PSUM") as ptpool,
        tc.tile_pool(name="py", bufs=2, space="PSUM") as pypool,
        tc.tile_pool(name="pcorr", bufs=1, space="PSUM") as pcpool,
    ):
        corr_ps = pcpool.tile([64, 64], F32, name="corr_ps")
        drain_i = 0
        for b in range(NBLK):
            xb = xblks[b]
            # xt/yt: [w127, i_h, wh, (hh c)]
            xt = xtpool.tile([P, HB, 2, P], F32R, tag="xt")
            yt = ytpool.tile([P, HB, 2, P], F32R, tag="yt")

            # transposes: Xt[w, (hh c)] = x-slab^T (PE transpose mode)
            for g in range(HB * 2 // 4):  # groups of 4 -> one psum bank
                ps = ptpool.tile([P, 2, 2, P], F32R, tag="ps_t")
                for k in range(4):
                    idx = g * 4 + k
                    i_h, wh = divmod(idx, 2)
                    nc.tensor.transpose(
                        ps[:, k // 2, k % 2, :],
                        xb[:, i_h, bass.ts(wh, P)],
                        identr,
                    )
                eng = nc.scalar if (drain_i % 2) else nc.vector
                drain_i += 1
                cp = eng.copy if eng is nc.scalar else eng.tensor_copy
                cp(out=xt[:, 2 * g : 2 * g + 2, :, :], in_=ps)

            # x_angular^T: Yt[w', (i_h, hh c)] = sum_v A[v, w'] Xt[v, (i_h, hh c)]
            for whp in range(2):
                for ch in range(HB // 4):  # psum chunks of [128, 4*128]
                    ps = pypool.tile([P, 4, P], F32, tag="ps_y")
                    for vh in range(2):
                        nc.tensor.matmul(
                            ps,
                            lhsT=A_sbr[:, vh, bass.ts(whp, P)],
                            rhs=xt[:, 4 * ch : 4 * ch + 4, vh, :],
                            start=(vh == 0),
                            stop=(vh == 1),
                        )
                    eng = nc.scalar if (drain_i % 2) else nc.vector
                    drain_i += 1
                    cp = eng.copy if eng is nc.scalar else eng.tensor_copy
                    cp(out=yt[:, 4 * ch : 4 * ch + 4, whp, :], in_=ps)

            # corr^T[j, i] += Yt-chunk^T @ Xt-chunk
            for i_h in range(HB):
                for wh in range(2):
                    for hh in range(2):
                        nc.tensor.matmul(
                            corr_ps,
                            lhsT=yt[:, i_h, wh, hh * 64 : (hh + 1) * 64],
                            rhs=xt[:, i_h, wh, hh * 64 : (hh + 1) * 64],
                            start=(i_corr == 0),
                            stop=(i_corr == n_corr - 1),
                        )
                        i_corr += 1

            if dbg is not None and b == 0:
                nc.sync.dma_start(out=dbg["xt0"].ap(), in_=xt.bitcast(F32))
                nc.sync.dma_start(out=dbg["yt0"].ap(), in_=yt.bitcast(F32))

        # ---------------- softmax over free axis of corrT ----------------
        corr_sb = small.tile([64, 64], F32, name="corr_sb")
        nc.vector.tensor_copy(out=corr_sb, in_=corr_ps)
        if dbg is not None:
            nc.sync.dma_start(out=dbg["corr"].ap(), in_=corr_sb)

    negm = small.tile([64, 1], F32, name="negm")
    nc.vector.tensor_reduce(
        out=negm, in_=corr_sb, axis=mybir.AxisListType.X,
        op=mybir.AluOpType.max, negate=True,
    )
    proT = small.tile([64, 64], F32, name="proT")
    nc.scalar.activation(
        out=proT, in_=corr_sb, func=mybir.ActivationFunctionType.Exp,
        bias=negm, scale=1.0,
    )
    ssum = small.tile([64, 1], F32, name="ssum")
    nc.vector.reduce_sum(out=ssum, in_=proT, axis=mybir.AxisListType.X)
    rinv = small.tile([64, 1], F32, name="rinv")
    nc.vector.reciprocal(out=rinv, in_=ssum)
    nc.vector.tensor_scalar_mul(out=proT, in0=proT, scalar1=rinv)
    if dbg is not None:
        nc.sync.dma_start(out=dbg["proT"].ap(), in_=proT)

    # block-diagonal [proT 0; 0 proT] (stage-C stationary weights)
    proT_bd = consts.tile([P, P], F32R, name="proT_bd")
    nc.vector.memset(proT_bd.bitcast(F32), 0.0)
    nc.vector.tensor_copy(out=proT_bd[0:64, 0:64], in_=proT)
    # cross-partition move -> SBUF-to-SBUF DMA
    nc.sync.dma_start(out=proT_bd[64:128, 64:128], in_=proT_bd[0:64, 0:64])

    # ---------------- phase C: out = pro @ X ----------------
    OB = 4  # h'-pairs per out DMA batch
    with (
        tc.tile_pool(name="po", bufs=2, space="PSUM") as popool,
        tc.tile_pool(name="outb", bufs=2) as outpool,
    ):
        drain_i = 0
        for b in range(NBLK):
            xb = xblks[b]
            for ob in range(HB // 2 // OB):  # out batches within block
                osb = outpool.tile([P, OB, 2, W], F32, tag="osb")
                for q in range(OB):
                    hp0 = (ob * OB + q) * 2
                    ps = popool.tile([P, 2, W], F32, tag="ps_o")
                    nc.tensor.matmul(
                        ps,
                        lhsT=proT_bd,
                        rhs=xb[:, hp0 : hp0 + 2, :],
                        start=True,
                        stop=True,
                    )
                    eng = nc.scalar if (drain_i % 2) else nc.vector
                    drain_i += 1
                    cp = eng.copy if eng is nc.scalar else eng.tensor_copy
                    cp(out=osb[:, q, :, :], in_=ps)
                hpb = b * HB + ob * OB * 2
                for hh in range(HH):
                    nc.sync.dma_start(
                        out=out_ap[:, hh * HP + hpb : hh * HP + hpb + OB * 2, :],
                        in_=osb[hh * 64 : (hh + 1) * 64].rearrange(
                            "p q t w -> p (q t) w"
                        ),
                    )


_GRAPH_CACHE = {}


def _get_graph(angle: int, debug_probes: bool = False):
    key = (angle, debug_probes)
    if key in _GRAPH_CACHE:
        return _GRAPH_CACHE[key]
    nc = bacc.Bacc(
        "TRN2",
        target_bir_lowering=False,
        debug=False,
        enable_asserts=False,
        num_devices=N_CORES,
    )
    x_d = nc.dram_tensor("x", [C, H, W], F32, kind="ExternalInput")
    a_d = nc.dram_tensor("amat", [W, W], F32, kind="ExternalInput")
    out_d = nc.dram_tensor("out", [C, H, W], F32, kind="ExternalOutput")
    dbg = None
    if debug_probes:
        dbg = {
            "xt0": nc.dram_tensor("dbg_xt0", [P, HB, 2, P], F32, kind="ExternalOutput"),
            "yt0": nc.dram_tensor("dbg_yt0", [P, HB, 2, P], F32, kind="ExternalOutput"),
            "corr": nc.dram_tensor("dbg_corr", [64, 64], F32, kind="ExternalOutput"),
            "proT": nc.dram_tensor("dbg_proT", [64, 64], F32, kind="ExternalOutput"),
        }
    from contextlib import ExitStack

    with tile.TileContext(nc) as tc, ExitStack() as ctx:
        _kernel_body(nc, tc, x_d, a_d, out_d, ctx, dbg=dbg)
    nc.compile()
    _GRAPH_CACHE[key] = nc
    return nc


def _run(x_cls: np.ndarray, angle: int, **spmd_kwargs):
    x_cls = np.ascontiguousarray(np.asarray(x_cls, dtype=np.float32))
    assert x_cls.shape == (B, C, H, W), x_cls.shape
    A = _build_A(int(angle))
    nc = _get_graph(int(angle))
    in_maps = [{"x": x_cls[i], "amat": A} for i in range(N_CORES)]
    res = run_bass_kernel_spmd(nc, in_maps, core_ids=list(range(N_CORES)), **spmd_kwargs)
    out = np.stack([r["out"] for r in res.results], axis=0)
    return out, res


def kernel(x_cls, angle):
    out, _ = _run(x_cls, int(np.asarray(angle)))
    return out
